# revision 12
# baseline (speedup 1.0000x reference)
"""CKConv (SIREN-generated full-length causal conv1d) on 8 TRN2 NeuronCores.

Problem (hardcoded shapes):
  x  : (2, 64, 2048)  f32
  W1 : (1, 128), W2 : (128, 128), W3 : (128, 64*64) f32   (SIREN generator)
  out: (2, 64, 2048)  f32

Math: rel = linspace(-1,1,L); h = sin(30*(sin(30*rel@W1)@W2)); k = h@W3
      kernel[co,ci,l] = k[l, co*64+ci]; causal conv (cross-correlation) of
      left-padded x with the odd-length (left-zero-padded) kernel. Reduces to
        out[b,co,t] = sum_{ci} sum_{d=0}^{t} x[b,ci,t-d] * g[d,co,ci],
        g[d,co,ci] = k[2047-d, co*64+ci].

Sharding (SPMD, identical program on all 8 cores, per-core data differs):
  Lag-pair interleave: lag pair p = (2p, 2p+1); core c owns pairs p ≡ c (mod 8),
  q = 0..127 indexes its pairs (p = c + 8q).  Each core computes a partial
  out over ALL (b, co, t) using only its lags; host sums the 8 partials.
  Per-tile matmul counts are identical across cores (perfect balance), and the
  core-dependent lag offset 2c is baked into per-core INPUT data:
    - xsh[(e*64+ci), b, s] = x[b, ci, s - PAD - 2c - e]   (host-prepared shifts)
    - relc[e*128+q] = rel[2047 - 2c - 16q - e]            (positions this core
      needs; generator is pointwise over positions so order is free)
  Device per core:
    1. hT1[r,idx] = sin(30 * W1[r] * relc[idx]);  hT2 = sin(30 * W2.T-matmul)
    2. for co: k-slice matmuls -> GA[(e,ci), co, q] = g[2(c+8q)+e, co, ci]
       (two col-tiled matmuls write psum partitions 0-63 / 64-127 directly)
    3. conv: for (b, time-tile i): accumulate 32*(i+1) matmuls
       out_psum[co, j] += sum_{(e,ci)} GA[(e,ci),co,q] * xsh[(e,ci), b,
                                         PAD + 512*i - 16*q + j]
"""

import numpy as np

B = 2
CIN = 64
COUT = 64
HID = 128
L = 2048
OMEGA = 30.0
NCORES = 8
QP = 128          # lag pairs per core
PAD = 496         # left margin in the shifted-x buffer
SLEN = PAD + L    # 2544
TT = 512          # conv time tile (PSUM bank limit for fp32)
NT = L // TT      # 4

_CACHE = {}

# "f32r": fp32r matmuls, M=64 (safe, ~2e-4 err)
# "bf16": bf16 matmuls, batch-paired col-tiling M=128 (fast, ~4e-3 err)
import os as _os
MODE = _os.environ.get("CK_MODE", "f32r")


def _build_bass(reps=1):
    import concourse.bacc as bacc
    import concourse.mybir as mybir
    import concourse.tile as tile
    import concourse.bass as bass

    f32 = mybir.dt.float32
    f32r = mybir.dt.float32r
    bf16 = mybir.dt.bfloat16
    cdt = bf16 if MODE == "bf16" else f32r   # conv/kgen-output dtype
    Sin = mybir.ActivationFunctionType.Sin

    nc = bacc.Bacc("TRN2", target_bir_lowering=False, debug=False,
                   num_devices=NCORES)

    xsh_d = nc.dram_tensor("xsh", [128, B, SLEN], cdt, kind="ExternalInput")
    relc_d = nc.dram_tensor("relc", [1, 2 * QP], f32, kind="ExternalInput")
    w1_d = nc.dram_tensor("w1", [HID, 1], f32, kind="ExternalInput")
    w2_d = nc.dram_tensor("w2", [HID, HID], f32r, kind="ExternalInput")
    w3_d = nc.dram_tensor("w3", [HID, COUT * CIN], f32r, kind="ExternalInput")
    outp_d = nc.dram_tensor("outp", [B, COUT, L], f32, kind="ExternalOutput")

    with tile.TileContext(nc) as tc:
        with (
            tc.tile_pool(name="const", bufs=1) as const,
            tc.tile_pool(name="outs", bufs=4) as outs,
            tc.tile_pool(name="gpsum", bufs=2, space="PSUM") as gpsum,
            tc.tile_pool(name="kpsum", bufs=2, space="PSUM") as kpsum,
            tc.tile_pool(name="convp", bufs=4, space="PSUM") as convp,
        ):
          for _rep in range(reps):
            # ---- load constants / inputs ----
            XX = const.tile([128, B, SLEN], cdt)
            # split the 2.6MB load across queues
            nsplit = 4
            step = SLEN // nsplit
            for j in range(nsplit):
                lo = j * step
                hi = SLEN if j == nsplit - 1 else (j + 1) * step
                nc.sync.dma_start(out=XX[:, :, lo:hi], in_=xsh_d[:, :, lo:hi])

            RELB = const.tile([128, 2 * QP], f32)
            relb_src = bass.AP(tensor=relc_d, offset=0,
                               ap=[[0, 128], [1, 2 * QP]])
            nc.sync.dma_start(out=RELB, in_=relb_src)

            W1S = const.tile([HID, 1], f32)
            nc.sync.dma_start(out=W1S, in_=w1_d[:, :])
            W2S = const.tile([HID, HID], f32r)
            nc.sync.dma_start(out=W2S, in_=w2_d[:, :])
            W3S = const.tile([HID, COUT * CIN], f32r)
            nc.sync.dma_start(out=W3S, in_=w3_d[:, :])

            # ---- SIREN generator ----
            # w1/w2 arrive pre-scaled by OMEGA; ScalarE Sin needs args in
            # [-pi, pi].  Range-reduce with the fp32 magic-rounding trick:
            #   n  = round(a/2pi)  via  (a/2pi + 1.5*2^23) - 1.5*2^23
            #   a' = a - n*2pi     (|a| <= 30, so n exact and a' in [-pi,pi])
            TWO_PI = float(2 * np.pi)
            MAGIC = float(1.5 * 2 ** 23)
            mult = mybir.AluOpType.mult
            add = mybir.AluOpType.add
            sub = mybir.AluOpType.subtract

            ZERO = const.tile([128, 1], f32)
            nc.vector.memset(ZERO, 0.0)

            def sin_reduced(dst, a_src):
                nr = const.tile([128, 2 * QP], f32, tag="rrtmp")
                nc.vector.tensor_scalar(nr, a_src, 1.0 / TWO_PI, MAGIC,
                                        mult, add)
                n2 = const.tile([128, 2 * QP], f32, tag="rrtmp2")
                nc.vector.tensor_scalar(n2, nr, MAGIC, -TWO_PI, sub, mult)
                ar = const.tile([128, 2 * QP], f32, tag="rrtmp3")
                nc.vector.tensor_add(ar, a_src, n2)
                nc.scalar.activation(dst, ar, Sin, bias=ZERO[:, 0:1])

            HT1 = const.tile([128, 2 * QP], f32)
            nc.vector.tensor_scalar(HT1, RELB, W1S[:, 0:1], None, mult)
            HT1S = const.tile([128, 2 * QP], f32r)
            sin_reduced(HT1S, HT1)

            pg = gpsum.tile([128, 2 * QP], f32)
            nc.tensor.matmul(pg, W2S, HT1S,
                             start=True, stop=True)
            HT2 = const.tile([128, 2 * QP], f32r)
            sin_reduced(HT2, pg)

            # ---- generated kernel slices: GAF[(e,ci), co, e', q] ----
            # One N=256 matmul per co -> psum[0:64] = [ci, (e', q)]; DVE/ACT
            # copy into GAF top half; then SBUF->SBUF DMA duplicates the e'=1
            # block into partitions 64-127 at e'=0 slots, so the conv's
            # stationary read GAF[:, co_all, 0, q] sees (e=0 | e=1) stacked.
            GAF = const.tile([128, COUT, 2, QP], cdt)
            for co in range(COUT):
                pk = kpsum.tile([64, 2 * QP], f32)
                w3c = W3S[:, co * CIN:(co + 1) * CIN]
                nc.tensor.matmul(pk, w3c, HT2, start=True, stop=True)
                if co % 2 == 0:
                    nc.vector.tensor_copy(GAF[0:64, co, :, :], pk)
                else:
                    nc.scalar.copy(GAF[0:64, co, :, :], pk)
            for j in range(4):
                c0, c1 = 16 * j, 16 * (j + 1)
                nc.sync.dma_start(out=GAF[64:128, c0:c1, 0, :],
                                  in_=GAF[0:64, c0:c1, 1, :])

            # ---- conv ----
            if MODE == "bf16":
                # batch-paired col-tiling: b=0 -> psum partitions 0-63,
                # b=1 -> 64-127; the two col-groups run concurrently.
                for i in range(NT):
                    po = convp.tile([128, TT], f32)
                    nq = (QP // NT) * (i + 1)
                    for q in range(nq):
                        s0 = PAD + TT * i - 16 * q
                        g = GAF[:, :, 0, q]
                        nc.tensor.matmul(po[0:64, :], g,
                                         XX[:, 0, s0:s0 + TT],
                                         start=(q == 0), stop=(q == nq - 1),
                                         tile_position=(0, 0))
                        nc.tensor.matmul(po[64:128, :], g,
                                         XX[:, 1, s0:s0 + TT],
                                         start=(q == 0), stop=(q == nq - 1),
                                         tile_position=(0, 64))
                    ot = outs.tile([128, TT], f32)
                    nc.vector.tensor_copy(ot, po)
                    nc.sync.dma_start(out=outp_d[0, :, TT * i:TT * (i + 1)],
                                      in_=ot[0:64, :])
                    nc.sync.dma_start(out=outp_d[1, :, TT * i:TT * (i + 1)],
                                      in_=ot[64:128, :])
            else:
                for b in range(B):
                    for i in range(NT):
                        po = convp.tile([COUT, TT], f32)
                        nq = (QP // NT) * (i + 1)
                        for q in range(nq):
                            s0 = PAD + TT * i - 16 * q
                            nc.tensor.matmul(po, GAF[:, :, 0, q],
                                             XX[:, b, s0:s0 + TT],
                                             start=(q == 0), stop=(q == nq - 1))
                        ot = outs.tile([COUT, TT], f32)
                        nc.vector.tensor_copy(ot, po)
                        nc.sync.dma_start(out=outp_d[b, :, TT * i:TT * (i + 1)],
                                          in_=ot)

    nc.compile()
    return nc


def _host_shard(x, W1, W2, W3):
    """Build the 8 per-core input maps."""
    rel = np.linspace(-1.0, 1.0, L, dtype=np.float32)
    xt = np.ascontiguousarray(x.transpose(1, 0, 2))  # (ci, b, t)
    in_maps = []
    for c in range(NCORES):
        xsh = np.zeros((128, B, SLEN), dtype=np.float32)
        # top half: e=0 -> x[b, ci, s - PAD - 2c]
        xsh[0:64, :, PAD + 2 * c: SLEN] = xt[:, :, :L - 2 * c]
        # bottom half: e=1 -> x[b, ci, s - PAD - 2c - 1]
        xsh[64:128, :, PAD + 2 * c + 1: SLEN] = xt[:, :, :L - 2 * c - 1]
        if MODE == "bf16":
            import ml_dtypes
            xsh = xsh.astype(ml_dtypes.bfloat16)

        q = np.arange(QP)
        relc = np.empty((1, 2 * QP), dtype=np.float32)
        relc[0, 0:QP] = rel[2047 - 2 * c - 16 * q]          # e = 0
        relc[0, QP:2 * QP] = rel[2046 - 2 * c - 16 * q]     # e = 1
        in_maps.append({
            "xsh": xsh,
            "relc": relc,
            "w1": np.ascontiguousarray(OMEGA * W1.reshape(HID, 1)),
            "w2": np.ascontiguousarray(OMEGA * W2),
            "w3": np.ascontiguousarray(W3),
        })
    return in_maps


def kernel(x, W1, W2, W3):
    x = np.asarray(x, dtype=np.float32)
    W1 = np.asarray(W1, dtype=np.float32)
    W2 = np.asarray(W2, dtype=np.float32)
    W3 = np.asarray(W3, dtype=np.float32)

    if "nc" not in _CACHE:
        _CACHE["nc"] = _build_bass()
    nc = _CACHE["nc"]

    from concourse import bass_utils
    in_maps = _host_shard(x, W1, W2, W3)
    res = bass_utils.run_bass_kernel_spmd(nc, in_maps,
                                          core_ids=list(range(NCORES)))
    _CACHE["last_res"] = res
    out = np.zeros((B, COUT, L), dtype=np.float64)
    for r in res.results:
        out += r["outp"].astype(np.float64)
    return out.astype(np.float32)


# revision 13
# speedup vs baseline: 1.4183x; 1.4183x over previous
"""CKConv (SIREN-generated full-length causal conv1d) on 8 TRN2 NeuronCores.

Problem (hardcoded shapes):
  x  : (2, 64, 2048)  f32
  W1 : (1, 128), W2 : (128, 128), W3 : (128, 64*64) f32   (SIREN generator)
  out: (2, 64, 2048)  f32

Math: rel = linspace(-1,1,L); h = sin(30*(sin(30*rel@W1)@W2)); k = h@W3
      kernel[co,ci,l] = k[l, co*64+ci]; causal conv (cross-correlation) of
      left-padded x with the odd-length (left-zero-padded) kernel. Reduces to
        out[b,co,t] = sum_{ci} sum_{d=0}^{t} x[b,ci,t-d] * g[d,co,ci],
        g[d,co,ci] = k[2047-d, co*64+ci].

Sharding (SPMD, identical program on all 8 cores, per-core data differs):
  Lag-pair interleave: lag pair p = (2p, 2p+1); core c owns pairs p ≡ c (mod 8),
  q = 0..127 indexes its pairs (p = c + 8q).  Each core computes a partial
  out over ALL (b, co, t) using only its lags; host sums the 8 partials.
  Per-tile matmul counts are identical across cores (perfect balance), and the
  core-dependent lag offset 2c is baked into per-core INPUT data:
    - xsh[(e*64+ci), b, s] = x[b, ci, s - PAD - 2c - e]   (host-prepared shifts)
    - relc[e*128+q] = rel[2047 - 2c - 16q - e]            (positions this core
      needs; generator is pointwise over positions so order is free)
  Device per core:
    1. hT1[r,idx] = sin(30 * W1[r] * relc[idx]);  hT2 = sin(30 * W2.T-matmul)
    2. for co: k-slice matmuls -> GA[(e,ci), co, q] = g[2(c+8q)+e, co, ci]
       (two col-tiled matmuls write psum partitions 0-63 / 64-127 directly)
    3. conv: for (b, time-tile i): accumulate 32*(i+1) matmuls
       out_psum[co, j] += sum_{(e,ci)} GA[(e,ci),co,q] * xsh[(e,ci), b,
                                         PAD + 512*i - 16*q + j]
"""

import numpy as np

B = 2
CIN = 64
COUT = 64
HID = 128
L = 2048
OMEGA = 30.0
NCORES = 8
QP = 128          # lag pairs per core
PAD = 496         # left margin in the shifted-x buffer
SLEN = PAD + L    # 2544
TT = 512          # conv time tile (PSUM bank limit for fp32)
NT = L // TT      # 4

_CACHE = {}

# "f32r": fp32r matmuls, M=64 (safe, ~2e-4 err)
# "bf16": bf16 matmuls, batch-paired col-tiling M=128 (fast, ~4e-3 err)
import os as _os
MODE = _os.environ.get("CK_MODE", "f32r")


def _build_bass(reps=1, rep_scope=None):
    if rep_scope is None:
        rep_scope = _os.environ.get("CK_REP_SCOPE", "all")
    import concourse.bacc as bacc
    import concourse.mybir as mybir
    import concourse.tile as tile
    import concourse.bass as bass

    f32 = mybir.dt.float32
    f32r = mybir.dt.float32r
    bf16 = mybir.dt.bfloat16
    cdt = bf16 if MODE == "bf16" else f32r   # conv/kgen-output dtype
    Sin = mybir.ActivationFunctionType.Sin

    nc = bacc.Bacc("TRN2", target_bir_lowering=False, debug=False,
                   num_devices=NCORES)

    xsh_d = nc.dram_tensor("xsh", [128, B, SLEN], cdt, kind="ExternalInput")
    relc_d = nc.dram_tensor("relc", [1, 2 * QP], f32, kind="ExternalInput")
    w1_d = nc.dram_tensor("w1", [HID, 1], f32, kind="ExternalInput")
    w2_d = nc.dram_tensor("w2", [HID, HID], f32r, kind="ExternalInput")
    w3_d = nc.dram_tensor("w3", [HID, COUT * CIN], f32r, kind="ExternalInput")
    outp_d = nc.dram_tensor("outp", [B, COUT, L], f32, kind="ExternalOutput")

    with tile.TileContext(nc) as tc:
        with (
            tc.tile_pool(name="const", bufs=1) as const,
            tc.tile_pool(name="outs", bufs=4) as outs,
            tc.tile_pool(name="gpsum", bufs=2, space="PSUM") as gpsum,
            tc.tile_pool(name="kpsum", bufs=2, space="PSUM") as kpsum,
            tc.tile_pool(name="convp", bufs=4, space="PSUM") as convp,
        ):
          load_reps = reps if rep_scope == "all" else 1
          gen_reps = reps if rep_scope in ("all", "gen") else 1
          conv_reps = reps if rep_scope in ("all", "conv") else 1
          for _rep in range(load_reps):
            # ---- load constants / inputs ----
            XX = const.tile([128, B, SLEN], cdt)
            # split the 2.6MB load across queues
            nsplit = 4
            step = SLEN // nsplit
            for j in range(nsplit):
                lo = j * step
                hi = SLEN if j == nsplit - 1 else (j + 1) * step
                nc.sync.dma_start(out=XX[:, :, lo:hi], in_=xsh_d[:, :, lo:hi])

            RELB = const.tile([128, 2 * QP], f32)
            relb_src = bass.AP(tensor=relc_d, offset=0,
                               ap=[[0, 128], [1, 2 * QP]])
            nc.sync.dma_start(out=RELB, in_=relb_src)

            W1S = const.tile([HID, 1], f32)
            nc.sync.dma_start(out=W1S, in_=w1_d[:, :])
            W2S = const.tile([HID, HID], f32r)
            nc.sync.dma_start(out=W2S, in_=w2_d[:, :])
            W3S = const.tile([HID, COUT * CIN], f32r)
            nc.sync.dma_start(out=W3S, in_=w3_d[:, :])

          for _rep in range(gen_reps):
            # ---- SIREN generator ----
            # w1/w2 arrive pre-scaled by OMEGA; ScalarE Sin needs args in
            # [-pi, pi].  Range-reduce with the fp32 magic-rounding trick:
            #   n  = round(a/2pi)  via  (a/2pi + 1.5*2^23) - 1.5*2^23
            #   a' = a - n*2pi     (|a| <= 30, so n exact and a' in [-pi,pi])
            TWO_PI = float(2 * np.pi)
            MAGIC = float(1.5 * 2 ** 23)
            mult = mybir.AluOpType.mult
            add = mybir.AluOpType.add
            sub = mybir.AluOpType.subtract

            ZERO = const.tile([128, 1], f32)
            nc.vector.memset(ZERO, 0.0)

            def sin_reduced(dst, a_src):
                nr = const.tile([128, 2 * QP], f32, tag="rrtmp")
                nc.vector.tensor_scalar(nr, a_src, 1.0 / TWO_PI, MAGIC,
                                        mult, add)
                n2 = const.tile([128, 2 * QP], f32, tag="rrtmp2")
                nc.vector.tensor_scalar(n2, nr, MAGIC, -TWO_PI, sub, mult)
                ar = const.tile([128, 2 * QP], f32, tag="rrtmp3")
                nc.vector.tensor_add(ar, a_src, n2)
                nc.scalar.activation(dst, ar, Sin, bias=ZERO[:, 0:1])

            HT1 = const.tile([128, 2 * QP], f32)
            nc.vector.tensor_scalar(HT1, RELB, W1S[:, 0:1], None, mult)
            HT1S = const.tile([128, 2 * QP], f32r)
            sin_reduced(HT1S, HT1)

            pg = gpsum.tile([128, 2 * QP], f32)
            nc.tensor.matmul(pg, W2S, HT1S,
                             start=True, stop=True)
            HT2 = const.tile([128, 2 * QP], f32r)
            sin_reduced(HT2, pg)

            # ---- generated kernel slices: GAF[(e,ci), co, e', q] ----
            # One N=256 matmul per co -> psum[0:64] = [ci, (e', q)]; DVE/ACT
            # copy into GAF top half; then SBUF->SBUF DMA duplicates the e'=1
            # block into partitions 64-127 at e'=0 slots, so the conv's
            # stationary read GAF[:, co_all, 0, q] sees (e=0 | e=1) stacked.
            GAF = const.tile([128, COUT, 2, QP], cdt)
            for co in range(COUT):
                pk = kpsum.tile([64, 2 * QP], f32)
                w3c = W3S[:, co * CIN:(co + 1) * CIN]
                nc.tensor.matmul(pk, w3c, HT2, start=True, stop=True)
                if co % 2 == 0:
                    nc.vector.tensor_copy(GAF[0:64, co, :, :], pk)
                else:
                    nc.scalar.copy(GAF[0:64, co, :, :], pk)
            for j in range(4):
                c0, c1 = 16 * j, 16 * (j + 1)
                nc.sync.dma_start(out=GAF[64:128, c0:c1, 0, :],
                                  in_=GAF[0:64, c0:c1, 1, :])

          for _rep in range(conv_reps):
            # ---- conv ----
            if MODE == "bf16":
                # batch-paired col-tiling: b=0 -> psum partitions 0-63,
                # b=1 -> 64-127; the two col-groups run concurrently.
                for i in range(NT):
                    po = convp.tile([128, TT], f32)
                    nq = (QP // NT) * (i + 1)
                    for q in range(nq):
                        s0 = PAD + TT * i - 16 * q
                        g = GAF[:, :, 0, q]
                        nc.tensor.matmul(po[0:64, :], g,
                                         XX[:, 0, s0:s0 + TT],
                                         start=(q == 0), stop=(q == nq - 1),
                                         tile_position=(0, 0))
                        nc.tensor.matmul(po[64:128, :], g,
                                         XX[:, 1, s0:s0 + TT],
                                         start=(q == 0), stop=(q == nq - 1),
                                         tile_position=(0, 64))
                    ot = outs.tile([128, TT], f32)
                    nc.vector.tensor_copy(ot, po)
                    nc.sync.dma_start(out=outp_d[0, :, TT * i:TT * (i + 1)],
                                      in_=ot[0:64, :])
                    nc.sync.dma_start(out=outp_d[1, :, TT * i:TT * (i + 1)],
                                      in_=ot[64:128, :])
            else:
                for b in range(B):
                    for i in range(NT):
                        po = convp.tile([COUT, TT], f32)
                        nq = (QP // NT) * (i + 1)
                        for q in range(nq):
                            s0 = PAD + TT * i - 16 * q
                            nc.tensor.matmul(po, GAF[:, :, 0, q],
                                             XX[:, b, s0:s0 + TT],
                                             start=(q == 0), stop=(q == nq - 1))
                        ot = outs.tile([COUT, TT], f32)
                        nc.vector.tensor_copy(ot, po)
                        nc.sync.dma_start(out=outp_d[b, :, TT * i:TT * (i + 1)],
                                          in_=ot)

    nc.compile()
    return nc


def _host_shard(x, W1, W2, W3):
    """Build the 8 per-core input maps."""
    rel = np.linspace(-1.0, 1.0, L, dtype=np.float32)
    xt = np.ascontiguousarray(x.transpose(1, 0, 2))  # (ci, b, t)
    in_maps = []
    for c in range(NCORES):
        xsh = np.zeros((128, B, SLEN), dtype=np.float32)
        # top half: e=0 -> x[b, ci, s - PAD - 2c]
        xsh[0:64, :, PAD + 2 * c: SLEN] = xt[:, :, :L - 2 * c]
        # bottom half: e=1 -> x[b, ci, s - PAD - 2c - 1]
        xsh[64:128, :, PAD + 2 * c + 1: SLEN] = xt[:, :, :L - 2 * c - 1]
        if MODE == "bf16":
            import ml_dtypes
            xsh = xsh.astype(ml_dtypes.bfloat16)

        q = np.arange(QP)
        relc = np.empty((1, 2 * QP), dtype=np.float32)
        relc[0, 0:QP] = rel[2047 - 2 * c - 16 * q]          # e = 0
        relc[0, QP:2 * QP] = rel[2046 - 2 * c - 16 * q]     # e = 1
        in_maps.append({
            "xsh": xsh,
            "relc": relc,
            "w1": np.ascontiguousarray(OMEGA * W1.reshape(HID, 1)),
            "w2": np.ascontiguousarray(OMEGA * W2),
            "w3": np.ascontiguousarray(W3),
        })
    return in_maps


def kernel(x, W1, W2, W3):
    x = np.asarray(x, dtype=np.float32)
    W1 = np.asarray(W1, dtype=np.float32)
    W2 = np.asarray(W2, dtype=np.float32)
    W3 = np.asarray(W3, dtype=np.float32)

    if "nc" not in _CACHE:
        _CACHE["nc"] = _build_bass()
    nc = _CACHE["nc"]

    from concourse import bass_utils
    in_maps = _host_shard(x, W1, W2, W3)
    res = bass_utils.run_bass_kernel_spmd(nc, in_maps,
                                          core_ids=list(range(NCORES)))
    _CACHE["last_res"] = res
    out = np.zeros((B, COUT, L), dtype=np.float64)
    for r in res.results:
        out += r["outp"].astype(np.float64)
    return out.astype(np.float32)


# revision 14
# speedup vs baseline: 2.4083x; 1.6980x over previous
"""CKConv (SIREN-generated full-length causal conv1d) on 8 TRN2 NeuronCores.

Problem (hardcoded shapes):
  x  : (2, 64, 2048)  f32
  W1 : (1, 128), W2 : (128, 128), W3 : (128, 64*64) f32   (SIREN generator)
  out: (2, 64, 2048)  f32

Math: rel = linspace(-1,1,L); h = sin(30*(sin(30*rel@W1)@W2)); k = h@W3
      kernel[co,ci,l] = k[l, co*64+ci]; causal conv (cross-correlation) of
      left-padded x with the odd-length (left-zero-padded) kernel. Reduces to
        out[b,co,t] = sum_{ci} sum_{d=0}^{t} x[b,ci,t-d] * g[d,co,ci],
        g[d,co,ci] = k[2047-d, co*64+ci].

Sharding (SPMD, identical program on all 8 cores, per-core data differs):
  Lag-pair interleave: lag pair p = (2p, 2p+1); core c owns pairs p ≡ c (mod 8),
  q = 0..127 indexes its pairs (p = c + 8q).  Each core computes a partial
  out over ALL (b, co, t) using only its lags; host sums the 8 partials.
  Per-tile matmul counts are identical across cores (perfect balance), and the
  core-dependent lag offset 2c is baked into per-core INPUT data:
    - xsh[(e*64+ci), b, s] = x[b, ci, s - PAD - 2c - e]   (host-prepared shifts)
    - relc[e*128+q] = rel[2047 - 2c - 16q - e]            (positions this core
      needs; generator is pointwise over positions so order is free)
  Device per core:
    1. hT1[r,idx] = sin(30 * W1[r] * relc[idx]);  hT2 = sin(30 * W2.T-matmul)
    2. for co: k-slice matmuls -> GA[(e,ci), co, q] = g[2(c+8q)+e, co, ci]
       (two col-tiled matmuls write psum partitions 0-63 / 64-127 directly)
    3. conv: for (b, time-tile i): accumulate 32*(i+1) matmuls
       out_psum[co, j] += sum_{(e,ci)} GA[(e,ci),co,q] * xsh[(e,ci), b,
                                         PAD + 512*i - 16*q + j]
"""

import numpy as np

B = 2
CIN = 64
COUT = 64
HID = 128
L = 2048
OMEGA = 30.0
NCORES = 8
QP = 128          # lag pairs per core
PAD = 496         # left margin in the shifted-x buffer
SLEN = PAD + L    # 2544
TT = 512          # conv time tile (PSUM bank limit for fp32)
NT = L // TT      # 4

_CACHE = {}

# "f32r": fp32r matmuls, M=64 (safe, ~2e-4 err)
# "bf16": bf16 matmuls, batch-paired col-tiling M=128 (fast, ~4e-3 err)
import os as _os
MODE = _os.environ.get("CK_MODE", "f32r")


def _build_bass(reps=1, rep_scope=None):
    if rep_scope is None:
        rep_scope = _os.environ.get("CK_REP_SCOPE", "all")
    import concourse.bacc as bacc
    import concourse.mybir as mybir
    import concourse.tile as tile
    import concourse.bass as bass

    f32 = mybir.dt.float32
    f32r = mybir.dt.float32r
    bf16 = mybir.dt.bfloat16
    cdt = bf16 if MODE == "bf16" else f32r   # conv/kgen-output dtype
    Sin = mybir.ActivationFunctionType.Sin

    nc = bacc.Bacc("TRN2", target_bir_lowering=False, debug=False,
                   num_devices=NCORES)

    xsh_d = nc.dram_tensor("xsh", [128, B, SLEN], cdt, kind="ExternalInput")
    relc_d = nc.dram_tensor("relc", [1, 2 * QP], f32, kind="ExternalInput")
    w1_d = nc.dram_tensor("w1", [HID, 1], f32, kind="ExternalInput")
    w2_d = nc.dram_tensor("w2", [HID, HID], f32r, kind="ExternalInput")
    w3_d = nc.dram_tensor("w3", [HID, COUT * CIN], f32r, kind="ExternalInput")
    outp_d = nc.dram_tensor("outp", [B, COUT, L], f32, kind="ExternalOutput")

    with tile.TileContext(nc) as tc:
        with (
            tc.tile_pool(name="const", bufs=1) as const,
            tc.tile_pool(name="outs", bufs=4) as outs,
            tc.tile_pool(name="gpsum", bufs=2, space="PSUM") as gpsum,
            tc.tile_pool(name="kpsum", bufs=2, space="PSUM") as kpsum,
            tc.tile_pool(name="convp", bufs=4, space="PSUM") as convp,
        ):
          load_reps = reps if rep_scope == "all" else 1
          gen_reps = reps if rep_scope in ("all", "gen") else 1
          conv_reps = reps if rep_scope in ("all", "conv") else 1
          for _rep in range(load_reps):
            # ---- load constants / inputs ----
            XX = const.tile([128, B, SLEN], cdt)
            # split the 2.6MB load across queues
            nsplit = 4
            step = SLEN // nsplit
            for j in range(nsplit):
                lo = j * step
                hi = SLEN if j == nsplit - 1 else (j + 1) * step
                nc.sync.dma_start(out=XX[:, :, lo:hi], in_=xsh_d[:, :, lo:hi])

            RELB = const.tile([128, 2 * QP], f32)
            relb_src = bass.AP(tensor=relc_d, offset=0,
                               ap=[[0, 128], [1, 2 * QP]])
            nc.sync.dma_start(out=RELB, in_=relb_src)

            W1S = const.tile([HID, 1], f32)
            nc.sync.dma_start(out=W1S, in_=w1_d[:, :])
            W2S = const.tile([HID, HID], f32r)
            nc.sync.dma_start(out=W2S, in_=w2_d[:, :])
            W3S = const.tile([HID, COUT * CIN], f32r)
            nc.sync.dma_start(out=W3S, in_=w3_d[:, :])

          for _rep in range(gen_reps):
            # ---- SIREN generator ----
            # w1/w2 arrive pre-scaled by OMEGA; ScalarE Sin needs args in
            # [-pi, pi].  Range-reduce with the fp32 magic-rounding trick:
            #   n  = round(a/2pi)  via  (a/2pi + 1.5*2^23) - 1.5*2^23
            #   a' = a - n*2pi     (|a| <= 30, so n exact and a' in [-pi,pi])
            TWO_PI = float(2 * np.pi)
            MAGIC = float(1.5 * 2 ** 23)
            mult = mybir.AluOpType.mult
            add = mybir.AluOpType.add
            sub = mybir.AluOpType.subtract

            ZERO = const.tile([128, 1], f32)
            nc.vector.memset(ZERO, 0.0)

            def sin_reduced(dst, a_src):
                nr = const.tile([128, 2 * QP], f32, tag="rrtmp")
                nc.vector.tensor_scalar(nr, a_src, 1.0 / TWO_PI, MAGIC,
                                        mult, add)
                n2 = const.tile([128, 2 * QP], f32, tag="rrtmp2")
                nc.vector.tensor_scalar(n2, nr, MAGIC, -TWO_PI, sub, mult)
                ar = const.tile([128, 2 * QP], f32, tag="rrtmp3")
                nc.vector.tensor_add(ar, a_src, n2)
                nc.scalar.activation(dst, ar, Sin, bias=ZERO[:, 0:1])

            HT1 = const.tile([128, 2 * QP], f32)
            nc.vector.tensor_scalar(HT1, RELB, W1S[:, 0:1], None, mult)
            HT1S = const.tile([128, 2 * QP], f32r)
            sin_reduced(HT1S, HT1)

            pg = gpsum.tile([128, 2 * QP], f32)
            nc.tensor.matmul(pg, W2S, HT1S,
                             start=True, stop=True)
            HT2 = const.tile([128, 2 * QP], f32r)
            sin_reduced(HT2, pg)

            # ---- generated kernel slices: GATF[(e,ci), q, co] ----
            # One N=256 matmul per co -> psum[0:64] = [ci, (e', q)].  Strided
            # DVE/ACT copies scatter into [q, co]-major layout so the conv's
            # stationary read GATF[:, q, :] is CONTIGUOUS in co (a strided
            # stationary AP makes fp32r matmuls pathologically slow).  e'=0
            # goes to GATF top; e'=1 to staging GAE, then one SBUF->SBUF DMA
            # duplicates it into partitions 64-127.
            GATF = const.tile([128, QP, COUT], cdt)
            GAE = const.tile([64, QP, COUT], cdt)
            for co in range(COUT):
                pk = kpsum.tile([64, 2 * QP], f32)
                w3c = W3S[:, co * CIN:(co + 1) * CIN]
                nc.tensor.matmul(pk, w3c, HT2, start=True, stop=True)
                if co % 2 == 0:
                    nc.vector.tensor_copy(GATF[0:64, :, co], pk[:, 0:QP])
                    nc.scalar.copy(GAE[:, :, co], pk[:, QP:2 * QP])
                else:
                    nc.scalar.copy(GATF[0:64, :, co], pk[:, 0:QP])
                    nc.vector.tensor_copy(GAE[:, :, co], pk[:, QP:2 * QP])
            for j in range(4):
                q0, q1 = 32 * j, 32 * (j + 1)
                nc.sync.dma_start(out=GATF[64:128, q0:q1, :],
                                  in_=GAE[:, q0:q1, :])

          for _rep in range(conv_reps):
            # ---- conv ----
            if MODE == "bf16":
                # batch-paired col-tiling: b=0 -> psum partitions 0-63,
                # b=1 -> 64-127; the two col-groups run concurrently.
                for i in range(NT):
                    po = convp.tile([128, TT], f32)
                    nq = (QP // NT) * (i + 1)
                    for q in range(nq):
                        s0 = PAD + TT * i - 16 * q
                        g = GATF[:, q, :]
                        nc.tensor.matmul(po[0:64, :], g,
                                         XX[:, 0, s0:s0 + TT],
                                         start=(q == 0), stop=(q == nq - 1),
                                         tile_position=(0, 0))
                        nc.tensor.matmul(po[64:128, :], g,
                                         XX[:, 1, s0:s0 + TT],
                                         start=(q == 0), stop=(q == nq - 1),
                                         tile_position=(0, 64))
                    ot = outs.tile([128, TT], f32)
                    nc.vector.tensor_copy(ot, po)
                    nc.sync.dma_start(out=outp_d[0, :, TT * i:TT * (i + 1)],
                                      in_=ot[0:64, :])
                    nc.sync.dma_start(out=outp_d[1, :, TT * i:TT * (i + 1)],
                                      in_=ot[64:128, :])
            else:
                for b in range(B):
                    for i in range(NT):
                        po = convp.tile([COUT, TT], f32)
                        nq = (QP // NT) * (i + 1)
                        for q in range(nq):
                            s0 = PAD + TT * i - 16 * q
                            nc.tensor.matmul(po, GATF[:, q, :],
                                             XX[:, b, s0:s0 + TT],
                                             start=(q == 0), stop=(q == nq - 1))
                        ot = outs.tile([COUT, TT], f32)
                        nc.vector.tensor_copy(ot, po)
                        nc.sync.dma_start(out=outp_d[b, :, TT * i:TT * (i + 1)],
                                          in_=ot)

    nc.compile()
    return nc


def _host_shard(x, W1, W2, W3):
    """Build the 8 per-core input maps."""
    rel = np.linspace(-1.0, 1.0, L, dtype=np.float32)
    xt = np.ascontiguousarray(x.transpose(1, 0, 2))  # (ci, b, t)
    in_maps = []
    for c in range(NCORES):
        xsh = np.zeros((128, B, SLEN), dtype=np.float32)
        # top half: e=0 -> x[b, ci, s - PAD - 2c]
        xsh[0:64, :, PAD + 2 * c: SLEN] = xt[:, :, :L - 2 * c]
        # bottom half: e=1 -> x[b, ci, s - PAD - 2c - 1]
        xsh[64:128, :, PAD + 2 * c + 1: SLEN] = xt[:, :, :L - 2 * c - 1]
        if MODE == "bf16":
            import ml_dtypes
            xsh = xsh.astype(ml_dtypes.bfloat16)

        q = np.arange(QP)
        relc = np.empty((1, 2 * QP), dtype=np.float32)
        relc[0, 0:QP] = rel[2047 - 2 * c - 16 * q]          # e = 0
        relc[0, QP:2 * QP] = rel[2046 - 2 * c - 16 * q]     # e = 1
        in_maps.append({
            "xsh": xsh,
            "relc": relc,
            "w1": np.ascontiguousarray(OMEGA * W1.reshape(HID, 1)),
            "w2": np.ascontiguousarray(OMEGA * W2),
            "w3": np.ascontiguousarray(W3),
        })
    return in_maps


def kernel(x, W1, W2, W3):
    x = np.asarray(x, dtype=np.float32)
    W1 = np.asarray(W1, dtype=np.float32)
    W2 = np.asarray(W2, dtype=np.float32)
    W3 = np.asarray(W3, dtype=np.float32)

    if "nc" not in _CACHE:
        _CACHE["nc"] = _build_bass()
    nc = _CACHE["nc"]

    from concourse import bass_utils
    in_maps = _host_shard(x, W1, W2, W3)
    res = bass_utils.run_bass_kernel_spmd(nc, in_maps,
                                          core_ids=list(range(NCORES)))
    _CACHE["last_res"] = res
    out = np.zeros((B, COUT, L), dtype=np.float64)
    for r in res.results:
        out += r["outp"].astype(np.float64)
    return out.astype(np.float32)
